# revision 32
# baseline (speedup 1.0000x reference)
"""Trainium2 Bass kernel for nn_EncoderMemNN_14929306321427 (MemNN encoder).

Math (see reference.py): story (M=256, B=16, S=64) token ids; C (4, V, 128)
embedding tables. Per hop h: m_A = sum_S C[h][s], prob = softmax_M(m_A @ u),
m_C = sum_S C[h+1][s], u += prob @ m_C. u starts at 0, so hop-0's softmax is
uniform: C[0] is never needed and u after hop 0 is mean_M(E1).

Strategy: data-parallel over batch (2 rows/core, 8 cores, no collectives).
Host fuses tables 1..3 into ccat[V+1, 384] fp16 rows [C1|C2|C3] (row V = 0);
each token is ONE 768B dma_gather row. dma_gather indices are int16, so
tokens split between call A (table base, t<32768) and call B (base row
VB=17490, t>=VB); the overlap zone lets the host pick a per-group split KA so
every sentence contributes EXACTLY KA rows to A and 64-KA to B (zero-row
padding only for rare infeasible sentences). Each group's 64 slots are cut
into 4 equal pieces issued on the 4 SWDGE queues -- descriptor generation
(~8 ns/idx/queue, one Q7 core pair per queue) runs 4-way parallel. The
sentence-sum runs on the PE as identity-matmul accumulation into PSUM, two
768-col slots fused per matmul; the two PSUM sections are recombined by
ACT-copy + DVE-add straight into the [E1|E2|E3] layout the attention needs.
A small PE/ACT/DVE attention pipeline computes the 3 hops.
"""

import numpy as np

HOPS = 3
V = 50257
D = 128
M = 256
B = 16
S = 64
NCORES = 8
BL = B // NCORES            # batch rows per core
NS = BL * M                 # sentences per core
P = 128
NG = NS // P                # sentence groups of 128
DCAT = HOPS * D             # 384 f16 elems per fused row (768 B)
NEG = -100.0       # |logits| < 4, so exp(lg + NEG) underflows to 0 cleanly
IMAX = 32768                # call A covers idx 0..32767
VB = V + 1 - IMAX           # 17490: call B covers rows VB..V (V = zero row)
ZB = V - VB                 # 32767: B-call index of the zero row

_CACHE = {}


def _pieces(KA_g, KB_g):
    """Cut a group's KT slots into 4 near-equal queue pieces; a piece
    straddling the A/B boundary becomes two calls on the same queue. The
    queue-0 call blocks the GpSimd engine until generated, so issue it last."""
    KT = KA_g + KB_g
    sizes = [KT // 4 + (1 if i < KT % 4 else 0) for i in range(4)]
    cuts = np.cumsum([0] + sizes)
    specs = []
    for q in range(4):
        k0, k1 = int(cuts[q]), int(cuts[q + 1])
        if k1 <= KA_g:
            specs.append((q, k0, k1, True))
        elif k0 >= KA_g:
            specs.append((q, k0, k1, False))
        else:
            specs.append((q, k0, KA_g, True))
            specs.append((q, KA_g, k1, False))
    return [s for s in specs if s[0] != 0] + [s for s in specs if s[0] == 0]


def _blob_cols(KA, KB):
    """Column offsets of per-group idx sections + identg inside the blob."""
    offs, c = {}, 0
    for g in range(NG):
        offs[g, "a"] = c
        c += 8 * KA[g]
        offs[g, "b"] = c
        c += 8 * KB[g]
    offs["identg"] = c
    c += D
    return offs, c


def build(KA, KB, do_compile=True):
    from concourse import bacc, mybir, tile

    f32 = mybir.dt.float32
    f16 = mybir.dt.float16
    i16 = mybir.dt.int16
    Alu = mybir.AluOpType
    Act = mybir.ActivationFunctionType
    Ax = mybir.AxisListType

    offs, NB = _blob_cols(KA, KB)

    nc = bacc.Bacc(num_swdge_queues=4)
    ccat_d = nc.declare_dram_parameter("ccat", [V + 1, DCAT], f16, isOutput=False)
    blob_d = nc.declare_dram_parameter("blob", [P, NB], i16, isOutput=False)
    ident_d = nc.declare_dram_parameter("ident", [P, P], f32, isOutput=False)
    sel_d = nc.declare_dram_parameter("sel", [P, NG * 2], f32, isOutput=False)
    i2_d = nc.declare_dram_parameter("i2", [2, 2], f32, isOutput=False)
    mneg_d = nc.declare_dram_parameter("mneg", [BL, BL * M], f32, isOutput=False)
    # u is kept in column layout [d, b] on-device; host transposes the output
    out_d = nc.declare_dram_parameter("out", [D, BL], f32, isOutput=True)

    with tile.TileContext(nc) as tc:
        with (
            tc.tile_pool(name="const", bufs=1) as cpool,
            tc.tile_pool(name="gather", bufs=2) as gpool,
            tc.tile_pool(name="work", bufs=2) as wpool,
            tc.tile_pool(name="ps_e", bufs=2, space="PSUM") as ps_e,
            tc.tile_pool(name="ps_t", bufs=1, space="PSUM") as ps_t,
            tc.tile_pool(name="ps_col", bufs=1, space="PSUM") as ps_col,
            tc.tile_pool(name="ps_mm", bufs=1, space="PSUM") as ps_mm,
        ):
            # idx blob -- the first real gather only waits on this load
            blob = cpool.tile([P, NB], i16)
            nc.sync.dma_start(out=blob[:], in_=blob_d[:])
            identg = blob[:, offs["identg"]:offs["identg"] + D].bitcast(f16)
            ident = cpool.tile([P, P], f32)
            nc.sync.dma_start(out=ident[:], in_=ident_d[:])
            sel = cpool.tile([P, NG * 2], f32)
            nc.sync.dma_start(out=sel[:], in_=sel_d[:])
            i2 = cpool.tile([2, 2], f32)
            nc.sync.dma_start(out=i2[:], in_=i2_d[:])
            mneg = cpool.tile([BL, BL * M], f32)
            nc.sync.dma_start(out=mneg[:], in_=mneg_d[:])

            # E_all[p, g*384+d]: per-sentence sums [E1|E2|E3] for group g
            E_all = cpool.tile([P, NG * DCAT], f32)
            F1 = cpool.tile([P, NS], f32)
            F2 = cpool.tile([P, NS], f32)
            # hop-0 state, accumulated directly in column layout:
            # u0ps[d, b] = sum_g E1_g^T @ sel_g
            u0ps = ps_col.tile([P, BL], f32, tag="colT")

            for g in range(NG):
                KT = KA[g] + KB[g]
                # one tile per gather call: the identity-matmuls for a piece
                # start as soon as THAT piece's DMA lands instead of waiting
                # for the whole group
                pieces = _pieces(KA[g], KB[g])
                tiles = []
                for pi, (q, k0, k1, is_a) in enumerate(pieces):
                    gp = gpool.tile([P, k1 - k0, DCAT], f16, tag=f"gt{pi}")
                    if is_a:
                        in_ap = ccat_d[:]
                        c0 = offs[g, "a"] + 8 * k0
                        c1 = offs[g, "a"] + 8 * k1
                    else:
                        in_ap = ccat_d[VB:, :]
                        c0 = offs[g, "b"] + 8 * (k0 - KA[g])
                        c1 = offs[g, "b"] + 8 * (k1 - KA[g])
                    nc.gpsimd.dma_gather(
                        out_ap=gp[:], in_ap=in_ap,
                        idxs_ap=blob[:, c0:c1],
                        num_idxs=P * (k1 - k0), num_idxs_reg=P * (k1 - k0),
                        elem_size=DCAT, single_packet=False, queue_num=q,
                    )
                    tiles.append((gp, k1 - k0))
                # smallest piece first: under round-robin draining a piece
                # lands roughly in proportion to its size, and the PE consumes
                # in issue order -- this leaves only the largest piece's
                # matmuls after the last DMA byte lands
                tiles.sort(key=lambda t: t[1])

                # identity-matmul accumulation, 2 slots per matmul; a matmul
                # output must fit one PSUM bank (512 f32), so the 384-wide
                # rows are split into a [C1|C2] stream and a [C3] stream
                e12 = ps_e.tile([P, 4 * D], f32, tag="e12")
                e3 = ps_e.tile([P, 2 * D], f32, tag="e3")
                mms = [
                    (gp, 2 * t, min(2 * t + 2, sz))
                    for gp, sz in tiles
                    for t in range((sz + 1) // 2)
                ]
                for i, (gp, ks, ke) in enumerate(mms):
                    nc.tensor.matmul(
                        out=e12[:, 0:(ke - ks) * 2 * D],
                        lhsT=identg,
                        rhs=gp[:, ks:ke, 0:2 * D],
                        start=(i == 0), stop=(i == len(mms) - 1),
                    )
                    nc.tensor.matmul(
                        out=e3[:, 0:(ke - ks) * D],
                        lhsT=identg,
                        rhs=gp[:, ks:ke, 2 * D:DCAT],
                        start=(i == 0), stop=(i == len(mms) - 1),
                    )
                # combine the two fused-slot sections; only one tensor input
                # may read PSUM, so stage one section through SBUF on ACT
                eb = g * DCAT
                t1 = wpool.tile([P, 2 * D], f32, tag="sec1")
                nc.scalar.copy(out=t1[:], in_=e12[:, 2 * D:4 * D])
                nc.vector.tensor_tensor(
                    out=E_all[:, eb:eb + 2 * D], in0=e12[:, 0:2 * D],
                    in1=t1[:], op=Alu.add,
                )
                t2 = wpool.tile([P, D], f32, tag="sec3")
                nc.scalar.copy(out=t2[:], in_=e3[:, D:2 * D])
                nc.vector.tensor_tensor(
                    out=E_all[:, eb + 2 * D:eb + DCAT], in0=e3[:, 0:D],
                    in1=t2[:], op=Alu.add,
                )

                # hop 0: u = mean_M E1 (softmax of zero logits is uniform),
                # accumulated transposed so no row->column flip is needed
                nc.tensor.matmul(
                    out=u0ps[:], lhsT=E_all[:, eb:eb + D],
                    rhs=sel[:, g * 2:(g + 1) * 2],
                    start=(g == 0), stop=(g == NG - 1),
                )
                # F1/F2: E1^T and E2^T column blocks for the logits matmuls
                for t, F in ((0, F1), (1, F2)):
                    tp = ps_t.tile([P, P], f32, tag="tp")
                    nc.tensor.transpose(
                        out=tp[:], in_=E_all[:, eb + t * D:eb + (t + 1) * D],
                        identity=ident[:],
                    )
                    nc.scalar.copy(out=F[:, g * P:(g + 1) * P], in_=tp[:])


            u = wpool.tile([P, BL], f32, tag="u0")
            nc.scalar.activation(
                out=u[:], in_=u0ps[:], func=Act.Copy, scale=1.0 / M
            )

            # ---- hops 1..2 (u stays in column layout [d, b] throughout)
            for hop in (1, 2):
                F = F1 if hop == 1 else F2
                lg_ps = ps_mm.tile([BL, NS], f32, tag="mm")
                nc.tensor.matmul(out=lg_ps[:], lhsT=u[:], rhs=F[:], start=True, stop=True)
                lgm = wpool.tile([BL, NS], f32, tag="lgm")
                nc.vector.scalar_tensor_tensor(
                    out=lgm[:], in0=lg_ps[:], scalar=1.0, in1=mneg[:],
                    op0=Alu.mult, op1=Alu.add,
                )
                # |logits| < 4: skip the softmax max-subtraction entirely
                pe = wpool.tile([BL, NS], f32, tag="pe")
                den = wpool.tile([BL, 1], f32, tag="den")
                nc.scalar.activation(
                    out=pe[:], in_=lgm[:], func=Act.Exp, scale=1.0,
                    accum_out=den[:],
                )
                # normalize in row layout (rden is a per-partition scalar here)
                rden = wpool.tile([BL, 1], f32, tag="rden")
                nc.vector.reciprocal(out=rden[:], in_=den[:])
                pen = wpool.tile([BL, NS], f32, tag="pen")
                nc.vector.tensor_scalar(
                    out=pen[:], in0=pe[:], scalar1=rden[:], scalar2=None,
                    op0=Alu.mult,
                )
                # prob columns: ptall[:, 2g:2g+2] = pen[:, gP:(g+1)P]^T; all 4
                # transposes land in one PSUM tile, copied out in one op
                ptps = ps_col.tile([P, NG * BL], f32, tag="ptps")
                for g in range(NG):
                    nc.tensor.matmul(
                        out=ptps[:, g * BL:(g + 1) * BL],
                        lhsT=pen[:, g * P:(g + 1) * P], rhs=i2[:],
                        start=True, stop=True, skip_group_check=True,
                    )
                ptall = wpool.tile([P, NG * BL], f32, tag="ptall")
                nc.scalar.copy(out=ptall[:], in_=ptps[:])
                # o transposed: oT[d, b] = sum_g E_{hop+1,g}^T @ pt_g
                oT = ps_col.tile([P, BL], f32, tag="colT")
                for g in range(NG):
                    nc.tensor.matmul(
                        out=oT[:],
                        lhsT=E_all[:, g * DCAT + hop * D: g * DCAT + hop * D + D],
                        rhs=ptall[:, g * BL:(g + 1) * BL],
                        start=(g == 0), stop=(g == NG - 1),
                    )
                u2 = wpool.tile([P, BL], f32, tag=f"u{hop}")
                nc.vector.tensor_tensor(
                    out=u2[:], in0=oT[:], in1=u[:], op=Alu.add,
                )
                u = u2

            nc.sync.dma_start(out=out_d[:], in_=u[:])
    if do_compile:
        nc.compile()
    return nc


def _wrap16(idx):
    """flat [n] int16 -> SBUF layout [128, n//16]: value i at [i%16, i//16],
    replicated to the 8 16-partition groups the Q7 cores read."""
    n = idx.shape[0]
    w = np.zeros((16, n // 16), np.int16)
    w[np.arange(n) % 16, np.arange(n) // 16] = idx
    return np.tile(w, (8, 1))


def prep_inputs(story, C):
    """Host-side: fused fp16 table + balanced exact-count index layouts."""
    story = np.asarray(story)
    C = np.asarray(C, dtype=np.float32)
    s = story.transpose(1, 0, 2).astype(np.int32)       # (B, M, S)

    ccat = np.zeros((V + 1, DCAT), np.float16)
    ccat[:V] = np.concatenate([C[1], C[2], C[3]], axis=1).astype(np.float16)

    # per core: sort tokens per sentence, group sentences by nmin quartile;
    # pick per-group split KA so (almost) every sentence sends exactly KA
    # tokens to call A and 64-KA to call B
    per_core = []
    for i in range(NCORES):
        blk = np.sort(s[i * BL:(i + 1) * BL].reshape(NS, S), axis=1)
        own = np.repeat(np.arange(BL), M)
        nmin = (blk < VB).sum(1)
        nmax = (blk < IMAX).sum(1)
        order = np.argsort(nmin, kind="stable")
        groups = []
        for g in range(NG):
            pick = order[g * P:(g + 1) * P]
            groups.append((blk[pick], nmin[pick], nmax[pick], own[pick]))
        per_core.append(groups)

    KA = tuple(
        int(max(per_core[i][g][1].max() for i in range(NCORES)))
        for g in range(NG)
    )
    KB = []
    for g in range(NG):
        kb = 64 - KA[g]
        for i in range(NCORES):
            _, nmin, nmax, _ = per_core[i][g]
            a = np.clip(KA[g], nmin, nmax)
            kb = max(kb, int(64 - a.min()))
        KB.append(kb)
    KB = tuple(KB)

    ident = np.eye(P, dtype=np.float32)
    identg = np.eye(P, dtype=np.float16)
    i2 = np.eye(2, dtype=np.float32)
    offs, NB = _blob_cols(KA, KB)

    in_maps = []
    for i in range(NCORES):
        sel = np.zeros((P, NG * 2), np.float32)
        mneg = np.full((BL, BL * M), NEG, np.float32)
        blob = np.zeros((P, NB), np.int16)
        blob[:, offs["identg"]:offs["identg"] + D] = identg.view(np.int16)
        for g in range(NG):
            toks, nmin, nmax, owner = per_core[i][g]
            sel[np.arange(P), g * 2 + owner] = 1.0
            mneg[owner, g * P + np.arange(P)] = 0.0
            a = np.clip(KA[g], nmin, nmax)                  # (P,)
            ks = np.arange(KA[g])[:, None]                  # slot k, sentence p
            low = np.where(ks < a[None, :], toks.T[:KA[g]], 0).astype(np.int16)
            blob[:, offs[g, "a"]:offs[g, "a"] + 8 * KA[g]] = _wrap16(
                low.reshape(-1))
            # B slot j of sentence p holds token a[p]+j (shifted), else pad
            js = np.arange(KB[g])[:, None]                  # (KB, 1)
            src = np.minimum(a[None, :] + js, S - 1)
            high = np.where(
                js < (S - a)[None, :],
                np.take_along_axis(toks.T, src, axis=0).astype(np.int64) - VB,
                ZB,
            ).astype(np.int16)
            blob[:, offs[g, "b"]:offs[g, "b"] + 8 * KB[g]] = _wrap16(
                high.reshape(-1))
        in_maps.append({
            "ccat": ccat, "blob": blob, "ident": ident, "sel": sel,
            "i2": i2, "mneg": mneg,
        })
    return in_maps, KA, KB


def run(in_maps, KA, KB, trace=False, **kwargs):
    from concourse.bass_utils import run_bass_kernel_spmd

    key = (KA, KB)
    if key not in _CACHE:
        _CACHE[key] = build(KA, KB)
    nc = _CACHE[key]
    res = run_bass_kernel_spmd(
        nc, in_maps, core_ids=list(range(NCORES)), trace=trace, **kwargs
    )
    out = np.concatenate([r["out"].T for r in res.results], axis=0)
    return out, res


def kernel(story, C):
    in_maps, KA, KB = prep_inputs(story, C)
    out, _ = run(in_maps, KA, KB)
    return out.astype(np.float32)


# revision 34
# speedup vs baseline: 1.0863x; 1.0863x over previous
"""Trainium2 Bass kernel for nn_EncoderMemNN_14929306321427 (MemNN encoder).

Math (see reference.py): story (M=256, B=16, S=64) token ids; C (4, V, 128)
embedding tables. Per hop h: m_A = sum_S C[h][s], prob = softmax_M(m_A @ u),
m_C = sum_S C[h+1][s], u += prob @ m_C. u starts at 0, so hop-0's softmax is
uniform: C[0] is never needed and u after hop 0 is mean_M(E1).

Strategy: data-parallel over batch (2 rows/core, 8 cores, no collectives).
Host fuses tables 1..3 into ccat[V+1, 384] fp16 rows [C1|C2|C3] (row V = 0);
each token is ONE 768B dma_gather row. dma_gather indices are int16, so
tokens split between call A (table base, t<32768) and call B (base row
VB=17490, t>=VB); the overlap zone lets the host pick a per-group split KA so
every sentence contributes EXACTLY KA rows to A and 64-KA to B (zero-row
padding only for rare infeasible sentences). Each group's 64 slots are cut
into 4 equal pieces issued on the 4 SWDGE queues -- descriptor generation
(~8 ns/idx/queue, one Q7 core pair per queue) runs 4-way parallel. The
sentence-sum runs on the PE as identity-matmul accumulation into PSUM, two
768-col slots fused per matmul; the two PSUM sections are recombined by
ACT-copy + DVE-add straight into the [E1|E2|E3] layout the attention needs.
A small PE/ACT/DVE attention pipeline computes the 3 hops.
"""

import numpy as np

HOPS = 3
V = 50257
D = 128
M = 256
B = 16
S = 64
NCORES = 8
BL = B // NCORES            # batch rows per core
NS = BL * M                 # sentences per core
P = 128
NG = NS // P                # sentence groups of 128
DCAT = HOPS * D             # 384 f16 elems per fused row (768 B)
NEG = -100.0       # |logits| < 4, so exp(lg + NEG) underflows to 0 cleanly
IMAX = 32768                # call A covers idx 0..32767
VB = V + 1 - IMAX           # 17490: call B covers rows VB..V (V = zero row)
ZB = V - VB                 # 32767: B-call index of the zero row

_CACHE = {}


def _pieces(KA_g, KB_g):
    """Cut a group's KT slots into 4 near-equal queue pieces; a piece
    straddling the A/B boundary becomes two calls on the same queue. The
    queue-0 call blocks the GpSimd engine until generated, so issue it last."""
    KT = KA_g + KB_g
    sizes = [KT // 4 + (1 if i < KT % 4 else 0) for i in range(4)]
    cuts = np.cumsum([0] + sizes)
    specs = []
    for q in range(4):
        k0, k1 = int(cuts[q]), int(cuts[q + 1])
        if k1 <= KA_g:
            specs.append((q, k0, k1, True))
        elif k0 >= KA_g:
            specs.append((q, k0, k1, False))
        else:
            specs.append((q, k0, KA_g, True))
            specs.append((q, KA_g, k1, False))
    return [s for s in specs if s[0] != 0] + [s for s in specs if s[0] == 0]


def _blob_cols(KA, KB):
    """Column offsets of per-group idx sections + identg inside the blob."""
    offs, c = {}, 0
    for g in range(NG):
        offs[g, "a"] = c
        c += 8 * KA[g]
        offs[g, "b"] = c
        c += 8 * KB[g]
    offs["identg"] = c
    c += D
    return offs, c


def build(KA, KB, do_compile=True):
    from concourse import bacc, mybir, tile

    f32 = mybir.dt.float32
    f16 = mybir.dt.float16
    i16 = mybir.dt.int16
    Alu = mybir.AluOpType
    Act = mybir.ActivationFunctionType
    Ax = mybir.AxisListType

    offs, NB = _blob_cols(KA, KB)

    nc = bacc.Bacc(num_swdge_queues=4)
    ccat_d = nc.declare_dram_parameter("ccat", [V + 1, DCAT], f16, isOutput=False)
    blob_d = nc.declare_dram_parameter("blob", [P, NB], i16, isOutput=False)
    ident_d = nc.declare_dram_parameter("ident", [P, P], f32, isOutput=False)
    sel_d = nc.declare_dram_parameter("sel", [P, NG * 2], f32, isOutput=False)
    i2_d = nc.declare_dram_parameter("i2", [2, 2], f32, isOutput=False)
    mneg_d = nc.declare_dram_parameter("mneg", [BL, BL * M], f32, isOutput=False)
    # u is kept in column layout [d, b] on-device; host transposes the output
    out_d = nc.declare_dram_parameter("out", [D, BL], f32, isOutput=True)

    with tile.TileContext(nc) as tc:
        with (
            tc.tile_pool(name="const", bufs=1) as cpool,
            tc.tile_pool(name="gather", bufs=2) as gpool,
            tc.tile_pool(name="work", bufs=2) as wpool,
            tc.tile_pool(name="ps_e", bufs=2, space="PSUM") as ps_e,
            tc.tile_pool(name="ps_t", bufs=1, space="PSUM") as ps_t,
            tc.tile_pool(name="ps_col", bufs=1, space="PSUM") as ps_col,
            tc.tile_pool(name="ps_mm", bufs=1, space="PSUM") as ps_mm,
        ):
            # idx blob -- the first real gather only waits on this load
            blob = cpool.tile([P, NB], i16)
            nc.sync.dma_start(out=blob[:], in_=blob_d[:])
            identg = blob[:, offs["identg"]:offs["identg"] + D].bitcast(f16)
            ident = cpool.tile([P, P], f32)
            nc.sync.dma_start(out=ident[:], in_=ident_d[:])
            sel = cpool.tile([P, NG * 2], f32)
            nc.sync.dma_start(out=sel[:], in_=sel_d[:])
            i2 = cpool.tile([2, 2], f32)
            nc.sync.dma_start(out=i2[:], in_=i2_d[:])
            mneg = cpool.tile([BL, BL * M], f32)
            nc.sync.dma_start(out=mneg[:], in_=mneg_d[:])

            # E_all[p, g*384+d]: per-sentence sums [E1|E2|E3] for group g
            E_all = cpool.tile([P, NG * DCAT], f32)
            F1 = cpool.tile([P, NS], f32)
            F2 = cpool.tile([P, NS], f32)
            # hop-0 state, accumulated directly in column layout:
            # u0ps[d, b] = sum_g E1_g^T @ sel_g
            u0ps = ps_col.tile([P, BL], f32, tag="colT")

            for g in range(NG):
                KT = KA[g] + KB[g]
                # one tile per gather call: the identity-matmuls for a piece
                # start as soon as THAT piece's DMA lands instead of waiting
                # for the whole group
                pieces = _pieces(KA[g], KB[g])
                tiles = []
                for pi, (q, k0, k1, is_a) in enumerate(pieces):
                    gp = gpool.tile([P, k1 - k0, DCAT], f16, tag=f"gt{pi}")
                    if is_a:
                        in_ap = ccat_d[:]
                        c0 = offs[g, "a"] + 8 * k0
                        c1 = offs[g, "a"] + 8 * k1
                    else:
                        in_ap = ccat_d[VB:, :]
                        c0 = offs[g, "b"] + 8 * (k0 - KA[g])
                        c1 = offs[g, "b"] + 8 * (k1 - KA[g])
                    nc.gpsimd.dma_gather(
                        out_ap=gp[:], in_ap=in_ap,
                        idxs_ap=blob[:, c0:c1],
                        num_idxs=P * (k1 - k0), num_idxs_reg=P * (k1 - k0),
                        elem_size=DCAT, single_packet=False, queue_num=q,
                    )
                    tiles.append((gp, k1 - k0))
                # smallest piece first: under round-robin draining a piece
                # lands roughly in proportion to its size, and the PE consumes
                # in issue order -- this leaves only the largest piece's
                # matmuls after the last DMA byte lands
                tiles.sort(key=lambda t: t[1])

                # identity-matmul accumulation, 2 slots per matmul; a matmul
                # output must fit one PSUM bank (512 f32), so the 384-wide
                # rows are split into a [C1|C2] stream and a [C3] stream
                e12 = ps_e.tile([P, 4 * D], f32, tag="e12")
                e3 = ps_e.tile([P, 2 * D], f32, tag="e3")
                mms = [
                    (gp, 2 * t, min(2 * t + 2, sz))
                    for gp, sz in tiles
                    for t in range((sz + 1) // 2)
                ]
                for i, (gp, ks, ke) in enumerate(mms):
                    nc.tensor.matmul(
                        out=e12[:, 0:(ke - ks) * 2 * D],
                        lhsT=identg,
                        rhs=gp[:, ks:ke, 0:2 * D],
                        start=(i == 0), stop=(i == len(mms) - 1),
                    )
                    nc.tensor.matmul(
                        out=e3[:, 0:(ke - ks) * D],
                        lhsT=identg,
                        rhs=gp[:, ks:ke, 2 * D:DCAT],
                        start=(i == 0), stop=(i == len(mms) - 1),
                    )
                # combine the two fused-slot sections; only one tensor input
                # may read PSUM, so stage one section through SBUF on ACT
                eb = g * DCAT
                t1 = wpool.tile([P, 2 * D], f32, tag="sec1")
                nc.scalar.copy(out=t1[:], in_=e12[:, 2 * D:4 * D])
                nc.vector.tensor_tensor(
                    out=E_all[:, eb:eb + 2 * D], in0=e12[:, 0:2 * D],
                    in1=t1[:], op=Alu.add,
                )
                t2 = wpool.tile([P, D], f32, tag="sec3")
                nc.scalar.copy(out=t2[:], in_=e3[:, D:2 * D])
                nc.vector.tensor_tensor(
                    out=E_all[:, eb + 2 * D:eb + DCAT], in0=e3[:, 0:D],
                    in1=t2[:], op=Alu.add,
                )

                # hop 0: u = mean_M E1 (softmax of zero logits is uniform),
                # accumulated transposed so no row->column flip is needed
                nc.tensor.matmul(
                    out=u0ps[:], lhsT=E_all[:, eb:eb + D],
                    rhs=sel[:, g * 2:(g + 1) * 2],
                    start=(g == 0), stop=(g == NG - 1),
                )
                # F1/F2: E1^T and E2^T column blocks for the logits matmuls
                for t, F in ((0, F1), (1, F2)):
                    tp = ps_t.tile([P, P], f32, tag="tp")
                    nc.tensor.transpose(
                        out=tp[:], in_=E_all[:, eb + t * D:eb + (t + 1) * D],
                        identity=ident[:],
                    )
                    nc.scalar.copy(out=F[:, g * P:(g + 1) * P], in_=tp[:])


            u = wpool.tile([P, BL], f32, tag="u0")
            nc.scalar.activation(
                out=u[:], in_=u0ps[:], func=Act.Copy, scale=1.0 / M
            )

            # ---- hops 1..2 (u stays in column layout [d, b] throughout)
            for hop in (1, 2):
                F = F1 if hop == 1 else F2
                lg_ps = ps_mm.tile([BL, NS], f32, tag="mm")
                nc.tensor.matmul(out=lg_ps[:], lhsT=u[:], rhs=F[:], start=True, stop=True)
                lgm = wpool.tile([BL, NS], f32, tag="lgm")
                nc.vector.scalar_tensor_tensor(
                    out=lgm[:], in0=lg_ps[:], scalar=1.0, in1=mneg[:],
                    op0=Alu.mult, op1=Alu.add,
                )
                # |logits| < 4: skip the softmax max-subtraction entirely
                pe = wpool.tile([BL, NS], f32, tag="pe")
                den = wpool.tile([BL, 1], f32, tag="den")
                nc.scalar.activation(
                    out=pe[:], in_=lgm[:], func=Act.Exp, scale=1.0,
                    accum_out=den[:],
                )
                # normalize in row layout (rden is a per-partition scalar here)
                rden = wpool.tile([BL, 1], f32, tag="rden")
                nc.vector.reciprocal(out=rden[:], in_=den[:])
                pen = wpool.tile([BL, NS], f32, tag="pen")
                nc.vector.tensor_scalar(
                    out=pen[:], in0=pe[:], scalar1=rden[:], scalar2=None,
                    op0=Alu.mult,
                )
                # prob columns: ptall[:, 2g:2g+2] = pen[:, gP:(g+1)P]^T; all 4
                # transposes land in one PSUM tile, copied out in one op
                ptps = ps_col.tile([P, NG * BL], f32, tag="ptps")
                for g in range(NG):
                    nc.tensor.matmul(
                        out=ptps[:, g * BL:(g + 1) * BL],
                        lhsT=pen[:, g * P:(g + 1) * P], rhs=i2[:],
                        start=True, stop=True, skip_group_check=True,
                    )
                ptall = wpool.tile([P, NG * BL], f32, tag="ptall")
                nc.scalar.copy(out=ptall[:], in_=ptps[:])
                # o transposed: oT[d, b] = sum_g E_{hop+1,g}^T @ pt_g
                oT = ps_col.tile([P, BL], f32, tag="colT")
                for g in range(NG):
                    nc.tensor.matmul(
                        out=oT[:],
                        lhsT=E_all[:, g * DCAT + hop * D: g * DCAT + hop * D + D],
                        rhs=ptall[:, g * BL:(g + 1) * BL],
                        start=(g == 0), stop=(g == NG - 1),
                    )
                u2 = wpool.tile([P, BL], f32, tag=f"u{hop}")
                nc.vector.tensor_tensor(
                    out=u2[:], in0=oT[:], in1=u[:], op=Alu.add,
                )
                u = u2

            nc.sync.dma_start(out=out_d[:], in_=u[:])
    if do_compile:
        nc.compile()
    return nc


def _wrap16(idx):
    """flat [n] int16 -> SBUF layout [128, n//16]: value i at [i%16, i//16],
    replicated to the 8 16-partition groups the Q7 cores read."""
    n = idx.shape[0]
    w = np.zeros((16, n // 16), np.int16)
    w[np.arange(n) % 16, np.arange(n) // 16] = idx
    return np.tile(w, (8, 1))


def prep_inputs(story, C):
    """Host-side: fused fp16 table + balanced exact-count index layouts."""
    story = np.asarray(story)
    C = np.asarray(C, dtype=np.float32)
    s = story.transpose(1, 0, 2).astype(np.int32)       # (B, M, S)

    ccat = np.zeros((V + 1, DCAT), np.float16)
    ccat[:V] = np.concatenate([C[1], C[2], C[3]], axis=1).astype(np.float16)

    # per core: sort tokens per sentence, group sentences by nmin quartile;
    # pick per-group split KA so (almost) every sentence sends exactly KA
    # tokens to call A and 64-KA to call B
    per_core = []
    for i in range(NCORES):
        blk = np.sort(s[i * BL:(i + 1) * BL].reshape(NS, S), axis=1)
        own = np.repeat(np.arange(BL), M)
        nmin = (blk < VB).sum(1)
        nmax = (blk < IMAX).sum(1)
        order = np.argsort(nmin, kind="stable")
        groups = []
        for g in range(NG):
            pick = order[g * P:(g + 1) * P]
            groups.append((blk[pick], nmin[pick], nmax[pick], own[pick]))
        per_core.append(groups)

    KA = tuple(
        int(max(per_core[i][g][1].max() for i in range(NCORES)))
        for g in range(NG)
    )
    KB = []
    for g in range(NG):
        kb = 64 - KA[g]
        for i in range(NCORES):
            _, nmin, nmax, _ = per_core[i][g]
            a = np.clip(KA[g], nmin, nmax)
            kb = max(kb, int(64 - a.min()))
        KB.append(kb)
    KB = tuple(KB)

    ident = np.eye(P, dtype=np.float32)
    identg = np.eye(P, dtype=np.float16)
    i2 = np.eye(2, dtype=np.float32)
    offs, NB = _blob_cols(KA, KB)

    in_maps = []
    for i in range(NCORES):
        sel = np.zeros((P, NG * 2), np.float32)
        mneg = np.full((BL, BL * M), NEG, np.float32)
        blob = np.zeros((P, NB), np.int16)
        blob[:, offs["identg"]:offs["identg"] + D] = identg.view(np.int16)
        for g in range(NG):
            toks, nmin, nmax, owner = per_core[i][g]
            sel[np.arange(P), g * 2 + owner] = 1.0
            mneg[owner, g * P + np.arange(P)] = 0.0
            a = np.clip(KA[g], nmin, nmax)                  # (P,)
            ks = np.arange(KA[g])[:, None]                  # slot k, sentence p
            low = np.where(ks < a[None, :], toks.T[:KA[g]], 0).astype(np.int16)
            blob[:, offs[g, "a"]:offs[g, "a"] + 8 * KA[g]] = _wrap16(
                low.reshape(-1))
            # B slot j of sentence p holds token a[p]+j (shifted), else pad
            js = np.arange(KB[g])[:, None]                  # (KB, 1)
            src = np.minimum(a[None, :] + js, S - 1)
            high = np.where(
                js < (S - a)[None, :],
                np.take_along_axis(toks.T, src, axis=0).astype(np.int64) - VB,
                ZB,
            ).astype(np.int16)
            blob[:, offs[g, "b"]:offs[g, "b"] + 8 * KB[g]] = _wrap16(
                high.reshape(-1))
        in_maps.append({
            "ccat": ccat, "blob": blob, "ident": ident, "sel": sel,
            "i2": i2, "mneg": mneg,
        })
    return in_maps, KA, KB


def run(in_maps, KA, KB, trace=False, **kwargs):
    from concourse.bass_utils import run_bass_kernel_spmd

    key = (KA, KB)
    if key not in _CACHE:
        _CACHE[key] = build(KA, KB)
    nc = _CACHE[key]
    res = run_bass_kernel_spmd(
        nc, in_maps, core_ids=list(range(NCORES)), trace=trace, **kwargs
    )
    out = np.concatenate([r["out"].T for r in res.results], axis=0)
    return out, res


def kernel(story, C):
    in_maps, KA, KB = prep_inputs(story, C)
    out, _ = run(in_maps, KA, KB)
    return out.astype(np.float32)
